# revision 1
# baseline (speedup 1.0000x reference)
"""Trainium2 Bass kernel for the e3nn-style concat + per-irrep Linear problem.

Reference computation (N = 200000 nodes, 480-dim features per input):
  per input: 128x0e (dims 0:128) + 64x1e (dims 128:320) + 32x2e (dims 320:480)
  s = [s1, s2] @ W0 * inv0 + b0                   # [N, 128]
  v = einsum('nmi,mo->noi', [v1,v2], W1) * inv1   # [N, 64, 3]
  t = einsum('nmi,mo->noi', [t1,t2], W2) * inv2   # [N, 32, 5]
  out = concat([s, v.flat, t.flat], axis=1)       # [N, 480]

Strategy (memory-bound, data-parallel over nodes across 8 cores):
  - Host: repack both inputs into one channel-major tensor XR [1024, N]
    (irrep components de-interleaved into eight 128-row contraction slabs),
    fold the 1/sqrt(K) norms into the weights, pair up the five l=2
    components into block-diagonal weights.
  - Device: per 512-node block, two 1 MB chunked DMA loads, 8 fp32 matmuls
    with stationary weights (streaming 512 node-columns into PSUM),
    bias-add/copy PSUM->SBUF, one 1 MB chunked DMA store of the
    channel-major output outT [512, N].
  - Host: transpose/interleave outT back to the reference layout.
"""

import numpy as np

MUL0, MUL1, MUL2 = 128, 64, 32
N_TOTAL = 200000
N_CORES = 8
NC_NODES = N_TOTAL // N_CORES          # 25000
NODE_BLOCK = 512
N_BLOCKS = (NC_NODES + NODE_BLOCK - 1) // NODE_BLOCK   # 49
NPAD = N_BLOCKS * NODE_BLOCK           # 25088

_PROGRAM_CACHE = {}


def _build_program(npad):
    import concourse.mybir as mybir
    from concourse import bacc
    import concourse.tile as tile

    f32 = mybir.dt.float32
    NB = NODE_BLOCK
    nc = bacc.Bacc("TRN2", target_bir_lowering=False, debug=False)

    nblocks = npad // NODE_BLOCK
    xin = nc.dram_tensor("xin", [nblocks, 960, NODE_BLOCK], f32, kind="ExternalInput").ap()
    w0a = nc.dram_tensor("w0a", [128, 128], f32, kind="ExternalInput").ap()
    w0b = nc.dram_tensor("w0b", [128, 128], f32, kind="ExternalInput").ap()
    w1d = nc.dram_tensor("w1d", [128, 64], f32, kind="ExternalInput").ap()
    w2p = nc.dram_tensor("w2p", [128, 64], f32, kind="ExternalInput").ap()
    w2s = nc.dram_tensor("w2s", [64, 32], f32, kind="ExternalInput").ap()
    b0d = nc.dram_tensor("b0d", [128, 1], f32, kind="ExternalInput").ap()
    outT = nc.dram_tensor("outT", [nblocks, 480, NODE_BLOCK], f32, kind="ExternalOutput").ap()

    with tile.TileContext(nc) as tc:
        with (
            tc.tile_pool(name="wpool", bufs=1) as wpool,
            tc.tile_pool(name="inpool", bufs=6) as inpool,
            tc.tile_pool(name="psum", bufs=2, space="PSUM") as psum,
            tc.tile_pool(name="outpool", bufs=5) as outpool,
        ):
            wa_t = wpool.tile([128, 128], f32)
            wb_t = wpool.tile([128, 128], f32)
            w1_t = wpool.tile([128, 64], f32)
            w2p_t = wpool.tile([128, 64], f32)
            w2s_t = wpool.tile([64, 32], f32)
            b0_t = wpool.tile([128, 1], f32)
            # Weights ride the SWDGE ring so the HWDGE rings start streaming
            # block loads immediately.
            nc.gpsimd.dma_start(wa_t[:], w0a)
            nc.gpsimd.dma_start(wb_t[:], w0b)
            nc.gpsimd.dma_start(w1_t[:], w1d)
            nc.gpsimd.dma_start(w2p_t[:], w2p)
            nc.gpsimd.dma_start(w2s_t[:], w2s)
            nc.gpsimd.dma_start(b0_t[:], b0d)

            for blk in range(nblocks):
                # Split each block's load across both HWDGE rings (qSP gets
                # slabs [s1,s2,v0,v1]; qACT gets [v2,tp0,tp1] + [t4]) so the
                # two rings stream concurrently and loads never queue behind
                # stores. Stores go through SWDGE (gpsimd). The block-major
                # DRAM layout keeps each DMA inside one contiguous ~1.9 MB
                # region for HBM locality.
                tina = inpool.tile([128, 4 * NB], f32)
                tinb = inpool.tile([128, 3 * NB], f32)
                t4t = inpool.tile([64, NB], f32)
                nc.sync.dma_start(
                    tina[:].rearrange("p (c n) -> p c n", c=4),
                    xin[blk, 0:512, :].rearrange("(c p) n -> p c n", p=128),
                )
                nc.scalar.dma_start(
                    tinb[:].rearrange("p (c n) -> p c n", c=3),
                    xin[blk, 512:896, :].rearrange("(c p) n -> p c n", p=128),
                )
                nc.scalar.dma_start(t4t[:], xin[blk, 896:960, :])

                # out^T row chunks: [s(128)] [v0|v1] [v2|t0,t1] [t2,t3|t4,junk]
                p0 = psum.tile([128, NB], f32)
                p1 = psum.tile([128, NB], f32)
                p2 = psum.tile([128, NB], f32)
                p3 = psum.tile([96, NB], f32)

                nc.tensor.matmul(p0[:], wa_t[:], tina[:, 0:NB], start=True, stop=False)
                nc.tensor.matmul(p0[:], wb_t[:], tina[:, NB:2 * NB], start=False, stop=True)
                nc.tensor.matmul(p1[0:64, :], w1_t[:], tina[:, 2 * NB:3 * NB])
                nc.tensor.matmul(p1[64:128, :], w1_t[:], tina[:, 3 * NB:4 * NB])
                nc.tensor.matmul(p2[0:64, :], w1_t[:], tinb[:, 0:NB])
                nc.tensor.matmul(p2[64:128, :], w2p_t[:], tinb[:, NB:2 * NB])
                nc.tensor.matmul(p3[0:64, :], w2p_t[:], tinb[:, 2 * NB:3 * NB])
                nc.tensor.matmul(p3[64:96, :], w2s_t[:], t4t[:])

                tout = outpool.tile([128, 4 * NB], f32)
                nc.vector.tensor_scalar_add(tout[:, 0:NB], p0[:], b0_t[:])
                nc.vector.tensor_copy(tout[:, NB:2 * NB], p1[:])
                nc.vector.tensor_copy(tout[:, 2 * NB:3 * NB], p2[:])
                nc.vector.tensor_copy(tout[0:96, 3 * NB:4 * NB], p3[:])

                nc.gpsimd.dma_start(
                    outT[blk, 0:384, :].rearrange("(c p) n -> p c n", p=128),
                    tout[:, 0:3 * NB].rearrange("p (c n) -> p c n", c=3),
                )
                nc.gpsimd.dma_start(outT[blk, 384:480, :], tout[0:96, 3 * NB:4 * NB])

    nc.compile()
    return nc


def _get_program(npad):
    if npad not in _PROGRAM_CACHE:
        _PROGRAM_CACHE[npad] = _build_program(npad)
    return _PROGRAM_CACHE[npad]


def _repack_inputs(x1, x2):
    """Build XR [960, N]: channel-major, component-de-interleaved, both inputs.

    Row slabs: [s1] [s2] [v1_0|v2_0] [v1_1|v2_1] [v1_2|v2_2]
    [t_0|t_1] [t_2|t_3] [t_4], each t_i = [t1_i(32); t2_i(32)].
    """
    n = x1.shape[0]
    xr = np.empty((960, n), dtype=np.float32)
    xr[0:128] = x1[:, 0:128].T
    xr[128:256] = x2[:, 0:128].T
    v1 = x1[:, 128:320].reshape(n, MUL1, 3)
    v2 = x2[:, 128:320].reshape(n, MUL1, 3)
    for i in range(3):
        base = 256 + 128 * i
        xr[base:base + 64] = v1[:, :, i].T
        xr[base + 64:base + 128] = v2[:, :, i].T
    t1 = x1[:, 320:480].reshape(n, MUL2, 5)
    t2 = x2[:, 320:480].reshape(n, MUL2, 5)
    for i in range(5):
        base = 640 + 64 * i
        xr[base:base + 32] = t1[:, :, i].T
        xr[base + 32:base + 64] = t2[:, :, i].T
    return xr


def _prepare_in_maps(x1, x2, W0, W1, W2, b0):
    x1 = np.asarray(x1, dtype=np.float32)
    x2 = np.asarray(x2, dtype=np.float32)
    inv0 = np.float32(1.0 / np.sqrt(2 * MUL0))
    inv1 = np.float32(1.0 / np.sqrt(2 * MUL1))
    inv2 = np.float32(1.0 / np.sqrt(2 * MUL2))
    w0s = np.asarray(W0, np.float32) * inv0                            # [256, 128]
    w1s = np.ascontiguousarray(np.asarray(W1, np.float32) * inv1)      # [128, 64]
    w2s = np.ascontiguousarray(np.asarray(W2, np.float32) * inv2)      # [64, 32]
    w2pair = np.zeros((128, 64), dtype=np.float32)                     # blockdiag(W2s, W2s)
    w2pair[0:64, 0:32] = w2s
    w2pair[64:128, 32:64] = w2s
    weights = {
        "w0a": np.ascontiguousarray(w0s[0:128]),
        "w0b": np.ascontiguousarray(w0s[128:256]),
        "w1d": w1s,
        "w2p": w2pair,
        "w2s": w2s,
        "b0d": np.ascontiguousarray(np.asarray(b0, np.float32).reshape(128, 1)),
    }
    xr = _repack_inputs(x1, x2)
    in_maps = []
    for c in range(N_CORES):
        xrc = np.zeros((960, NPAD), dtype=np.float32)
        xrc[:, :NC_NODES] = xr[:, c * NC_NODES:(c + 1) * NC_NODES]
        # Block-major: [nblocks, 960, NODE_BLOCK], each block slab contiguous.
        xrb = np.ascontiguousarray(
            xrc.reshape(960, N_BLOCKS, NODE_BLOCK).transpose(1, 0, 2)
        )
        in_maps.append({"xin": xrb, **weights})
    return in_maps


def _assemble_output(outs):
    """outs: list of 8 outT arrays [nblocks, 480, NODE_BLOCK] -> [N_TOTAL, 480]."""
    full = np.empty((N_TOTAL, 480), dtype=np.float32)
    for c, o in enumerate(outs):
        o = o.transpose(1, 0, 2).reshape(480, NPAD)[:, :NC_NODES]
        rows = slice(c * NC_NODES, (c + 1) * NC_NODES)
        full[rows, 0:128] = o[0:128].T
        full[rows, 128:320] = (
            o[128:320].reshape(3, MUL1, NC_NODES).transpose(2, 1, 0).reshape(NC_NODES, 192)
        )
        full[rows, 320:480] = (
            o[320:480].reshape(5, MUL2, NC_NODES).transpose(2, 1, 0).reshape(NC_NODES, 160)
        )
    return full


def kernel(x1, x2, W0, W1, W2, b0):
    from concourse.bass_utils import run_bass_kernel_spmd

    in_maps = _prepare_in_maps(x1, x2, W0, W1, W2, b0)
    nc = _get_program(NPAD)
    res = run_bass_kernel_spmd(nc, in_maps, core_ids=list(range(N_CORES)))
    return _assemble_output([r["outT"] for r in res.results])



# revision 2
# speedup vs baseline: 1.6783x; 1.6783x over previous
"""Trainium2 Bass kernel for the e3nn-style concat + per-irrep Linear problem.

Reference computation (N = 200000 nodes, 480-dim features per input):
  per input: 128x0e (dims 0:128) + 64x1e (dims 128:320) + 32x2e (dims 320:480)
  s = [s1, s2] @ W0 * inv0 + b0                   # [N, 128]
  v = einsum('nmi,mo->noi', [v1,v2], W1) * inv1   # [N, 64, 3]
  t = einsum('nmi,mo->noi', [t1,t2], W2) * inv2   # [N, 32, 5]
  out = concat([s, v.flat, t.flat], axis=1)       # [N, 480]

Strategy (memory-bound, data-parallel over nodes across 8 cores):
  - Host: repack both inputs into channel-major fp16 (halves HBM read
    traffic vs fp32 and runs the PE at 4x the fp32 column rate), with the
    irrep components de-interleaved into 128-row contraction slabs and the
    1/sqrt(K) norms folded into the fp16 weights.
  - Device: per 1000-node block, two ~1 MB HWDGE loads (slabs 0-3 / 4-6),
    the small fifth l=2 component staged once up front, 16 fp16 matmuls
    (two 500-column halves), bias-add/copy PSUM->SBUF in fp16, SWDGE
    store of the channel-major fp16 output.
  - Host: transpose/interleave back to the reference fp32 layout.
"""

import numpy as np

MUL0, MUL1, MUL2 = 128, 64, 32
N_TOTAL = 200000
N_CORES = 8
NC_NODES = N_TOTAL // N_CORES          # 25000
NODE_BLOCK = 1000
N_BLOCKS = NC_NODES // NODE_BLOCK      # 25
HALF = NODE_BLOCK // 2                 # 500 (matmul N <= 512, one PSUM bank)

_PROGRAM_CACHE = {}


def _build_program():
    import concourse.mybir as mybir
    from concourse import bacc
    import concourse.tile as tile

    f16 = mybir.dt.float16
    f32 = mybir.dt.float32
    NB = NODE_BLOCK
    nc = bacc.Bacc("TRN2", target_bir_lowering=False, debug=False)

    nblocks = N_BLOCKS
    xin = nc.dram_tensor("xin", [nblocks, 896, NB], f16, kind="ExternalInput").ap()
    # t4 (fifth l=2 component, 64 rows) packed [128, NC_NODES/2]: node n<12500
    # on partitions 0:64 col n, else partitions 64:128 col n-12500.
    xt4 = nc.dram_tensor("xt4", [128, NC_NODES // 2], f16, kind="ExternalInput").ap()
    w0a = nc.dram_tensor("w0a", [128, 128], f16, kind="ExternalInput").ap()
    w0b = nc.dram_tensor("w0b", [128, 128], f16, kind="ExternalInput").ap()
    w1d = nc.dram_tensor("w1d", [128, 64], f16, kind="ExternalInput").ap()
    w2p = nc.dram_tensor("w2p", [128, 64], f16, kind="ExternalInput").ap()
    w2s = nc.dram_tensor("w2s", [128, 32], f16, kind="ExternalInput").ap()
    b0d = nc.dram_tensor("b0d", [128, 1], f32, kind="ExternalInput").ap()
    outT = nc.dram_tensor("outT", [nblocks, 480, NB], f16, kind="ExternalOutput").ap()

    with tile.TileContext(nc) as tc:
        with (
            tc.tile_pool(name="wpool", bufs=1) as wpool,
            tc.tile_pool(name="t4pool", bufs=1) as t4pool,
            tc.tile_pool(name="inpool", bufs=4) as inpool,
            tc.tile_pool(name="psum", bufs=2, space="PSUM") as psum,
            tc.tile_pool(name="outpool", bufs=3) as outpool,
        ):
            wa_t = wpool.tile([128, 128], f16)
            wb_t = wpool.tile([128, 128], f16)
            w1_t = wpool.tile([128, 64], f16)
            w2p_t = wpool.tile([128, 64], f16)
            w2s_t = wpool.tile([128, 32], f16)
            b0_t = wpool.tile([128, 1], f32)
            nc.gpsimd.dma_start(wa_t[:], w0a)
            nc.gpsimd.dma_start(wb_t[:], w0b)
            nc.gpsimd.dma_start(w1_t[:], w1d)
            nc.gpsimd.dma_start(w2p_t[:], w2p)
            nc.gpsimd.dma_start(w2s_t[:], w2s)
            nc.gpsimd.dma_start(b0_t[:], b0d)

            # Stage the whole t4 slab once (3.2 MB) on the store ring before
            # stores start flowing; per-block loads stream on the HWDGE rings.
            t4_t = t4pool.tile([128, NC_NODES // 2], f16)
            nc.gpsimd.dma_start(t4_t[:], xt4)

            for blk in range(nblocks):
                tina = inpool.tile([128, 4 * NB], f16)
                tinb = inpool.tile([128, 3 * NB], f16)
                nc.sync.dma_start(
                    tina[:].rearrange("p (c n) -> p c n", c=4),
                    xin[blk, 0:512, :].rearrange("(c p) n -> p c n", p=128),
                )
                nc.scalar.dma_start(
                    tinb[:].rearrange("p (c n) -> p c n", c=3),
                    xin[blk, 512:896, :].rearrange("(c p) n -> p c n", p=128),
                )

                tout = outpool.tile([128, 4 * NB], f16)
                for h in range(2):
                    lo = blk * NB + h * HALF          # node offset of this half
                    sl = slice(h * HALF, h * HALF + HALF)
                    # t4 columns for this half (see xt4 packing above)
                    t4c = lo % (NC_NODES // 2)
                    t4p = 0 if lo < NC_NODES // 2 else 64
                    t4_mv = t4_t[t4p:t4p + 64, t4c:t4c + HALF]

                    p0 = psum.tile([128, HALF], f32)
                    p1 = psum.tile([128, HALF], f32)
                    p2 = psum.tile([128, HALF], f32)
                    p3 = psum.tile([96, HALF], f32)

                    def a(c):  # column slice of slab c in tina
                        return tina[:, c * NB + h * HALF: c * NB + h * HALF + HALF]

                    def b(c):
                        return tinb[:, c * NB + h * HALF: c * NB + h * HALF + HALF]

                    nc.tensor.matmul(p0[:], wa_t[:], a(0), start=True, stop=False)
                    nc.tensor.matmul(p0[:], wb_t[:], a(1), start=False, stop=True)
                    nc.tensor.matmul(p1[0:64, :], w1_t[:], a(2))
                    nc.tensor.matmul(p1[64:128, :], w1_t[:], a(3))
                    nc.tensor.matmul(p2[0:64, :], w1_t[:], b(0))
                    nc.tensor.matmul(p2[64:128, :], w2p_t[:], b(1))
                    nc.tensor.matmul(p3[0:64, :], w2p_t[:], b(2))
                    nc.tensor.matmul(p3[64:96, :], w2s_t[t4p:t4p + 64, :], t4_mv)

                    nc.vector.tensor_scalar_add(tout[:, sl], p0[:], b0_t[:])
                    nc.vector.tensor_copy(
                        tout[:, NB + h * HALF: NB + h * HALF + HALF], p1[:])
                    nc.vector.tensor_copy(
                        tout[:, 2 * NB + h * HALF: 2 * NB + h * HALF + HALF], p2[:])
                    nc.vector.tensor_copy(
                        tout[0:96, 3 * NB + h * HALF: 3 * NB + h * HALF + HALF], p3[:])

                nc.gpsimd.dma_start(
                    outT[blk, 0:384, :].rearrange("(c p) n -> p c n", p=128),
                    tout[:, 0:3 * NB].rearrange("p (c n) -> p c n", c=3),
                )
                nc.gpsimd.dma_start(outT[blk, 384:480, :], tout[0:96, 3 * NB:4 * NB])

    nc.compile()
    return nc


def _get_program():
    if "p" not in _PROGRAM_CACHE:
        _PROGRAM_CACHE["p"] = _build_program()
    return _PROGRAM_CACHE["p"]


def _repack_inputs(x1, x2):
    """Build XR [896, N] fp16 (slabs s1,s2,v0,v1,v2,tp0,tp1) + T4 [64, N] fp16.

    Row slabs of XR: [s1] [s2] [v1_0|v2_0] [v1_1|v2_1] [v1_2|v2_2]
    [t_0|t_1] [t_2|t_3]; each t_i = [t1_i(32); t2_i(32)]. T4 = t_4.
    """
    n = x1.shape[0]
    xr = np.empty((896, n), dtype=np.float16)
    xr[0:128] = x1[:, 0:128].T
    xr[128:256] = x2[:, 0:128].T
    v1 = x1[:, 128:320].reshape(n, MUL1, 3)
    v2 = x2[:, 128:320].reshape(n, MUL1, 3)
    for i in range(3):
        base = 256 + 128 * i
        xr[base:base + 64] = v1[:, :, i].T
        xr[base + 64:base + 128] = v2[:, :, i].T
    t1 = x1[:, 320:480].reshape(n, MUL2, 5)
    t2 = x2[:, 320:480].reshape(n, MUL2, 5)
    for i in range(4):
        base = 640 + 64 * i
        xr[base:base + 32] = t1[:, :, i].T
        xr[base + 32:base + 64] = t2[:, :, i].T
    t4 = np.empty((64, n), dtype=np.float16)
    t4[0:32] = t1[:, :, 4].T
    t4[32:64] = t2[:, :, 4].T
    return xr, t4


def _prepare_in_maps(x1, x2, W0, W1, W2, b0):
    x1 = np.asarray(x1, dtype=np.float32)
    x2 = np.asarray(x2, dtype=np.float32)
    inv0 = np.float32(1.0 / np.sqrt(2 * MUL0))
    inv1 = np.float32(1.0 / np.sqrt(2 * MUL1))
    inv2 = np.float32(1.0 / np.sqrt(2 * MUL2))
    w0s = np.asarray(W0, np.float32) * inv0                            # [256, 128]
    w1s = np.asarray(W1, np.float32) * inv1                            # [128, 64]
    w2s = np.asarray(W2, np.float32) * inv2                            # [64, 32]
    w2pair = np.zeros((128, 64), dtype=np.float32)                     # blockdiag(W2s, W2s)
    w2pair[0:64, 0:32] = w2s
    w2pair[64:128, 32:64] = w2s
    w2stack = np.concatenate([w2s, w2s], axis=0)                       # [128, 32]
    weights = {
        "w0a": np.ascontiguousarray(w0s[0:128]).astype(np.float16),
        "w0b": np.ascontiguousarray(w0s[128:256]).astype(np.float16),
        "w1d": w1s.astype(np.float16),
        "w2p": w2pair.astype(np.float16),
        "w2s": w2stack.astype(np.float16),
        "b0d": np.ascontiguousarray(np.asarray(b0, np.float32).reshape(128, 1)),
    }
    xr, t4 = _repack_inputs(x1, x2)
    in_maps = []
    half = NC_NODES // 2
    for c in range(N_CORES):
        xrc = xr[:, c * NC_NODES:(c + 1) * NC_NODES]
        # Block-major: [nblocks, 896, NODE_BLOCK], each block slab contiguous.
        xrb = np.ascontiguousarray(
            xrc.reshape(896, N_BLOCKS, NODE_BLOCK).transpose(1, 0, 2)
        )
        t4c = t4[:, c * NC_NODES:(c + 1) * NC_NODES]
        xt4 = np.ascontiguousarray(
            t4c.reshape(64, 2, half).transpose(1, 0, 2).reshape(128, half)
        )
        in_maps.append({"xin": xrb, "xt4": xt4, **weights})
    return in_maps


def _assemble_output(outs):
    """outs: list of 8 outT arrays [nblocks, 480, NODE_BLOCK] -> [N_TOTAL, 480]."""
    full = np.empty((N_TOTAL, 480), dtype=np.float32)
    for c, o in enumerate(outs):
        o = np.asarray(o, np.float32).transpose(1, 0, 2).reshape(480, NC_NODES)
        rows = slice(c * NC_NODES, (c + 1) * NC_NODES)
        full[rows, 0:128] = o[0:128].T
        full[rows, 128:320] = (
            o[128:320].reshape(3, MUL1, NC_NODES).transpose(2, 1, 0).reshape(NC_NODES, 192)
        )
        full[rows, 320:480] = (
            o[320:480].reshape(5, MUL2, NC_NODES).transpose(2, 1, 0).reshape(NC_NODES, 160)
        )
    return full


def kernel(x1, x2, W0, W1, W2, b0):
    from concourse.bass_utils import run_bass_kernel_spmd

    in_maps = _prepare_in_maps(x1, x2, W0, W1, W2, b0)
    nc = _get_program()
    res = run_bass_kernel_spmd(nc, in_maps, core_ids=list(range(N_CORES)))
    return _assemble_output([r["outT"] for r in res.results])


# revision 9
# speedup vs baseline: 1.8455x; 1.0996x over previous
"""Trainium2 Bass kernel for the e3nn-style concat + per-irrep Linear problem.

Reference computation (N = 200000 nodes, 480-dim features per input):
  per input: 128x0e (dims 0:128) + 64x1e (dims 128:320) + 32x2e (dims 320:480)
  s = [s1, s2] @ W0 * inv0 + b0                   # [N, 128]
  v = einsum('nmi,mo->noi', [v1,v2], W1) * inv1   # [N, 64, 3]
  t = einsum('nmi,mo->noi', [t1,t2], W2) * inv2   # [N, 32, 5]
  out = concat([s, v.flat, t.flat], axis=1)       # [N, 480]

Strategy (memory-bound, data-parallel over nodes across 8 cores):
  - Host: repack both inputs into channel-major fp16 (halves HBM read
    traffic vs fp32 and runs the PE at 4x the fp32 column rate), with the
    irrep components de-interleaved into 128-row contraction slabs and the
    1/sqrt(K) norms folded into the fp16 weights.
  - Device: per 1000-node block, two ~1 MB HWDGE loads (slabs 0-3 / 4-6),
    the small fifth l=2 component staged once up front, 16 fp16 matmuls
    (two 500-column halves), bias-add/copy PSUM->SBUF in fp16, SWDGE
    store of the channel-major fp16 output.
  - Host: transpose/interleave back to the reference fp32 layout.
"""

import numpy as np

MUL0, MUL1, MUL2 = 128, 64, 32
N_TOTAL = 200000
N_CORES = 8
NC_NODES = N_TOTAL // N_CORES          # 25000
NODE_BLOCK = 1000
N_BLOCKS = NC_NODES // NODE_BLOCK      # 25
HALF = NODE_BLOCK // 2                 # 500 (matmul N <= 512, one PSUM bank)

_PROGRAM_CACHE = {}


def _build_program():
    import concourse.mybir as mybir
    from concourse import bacc
    import concourse.tile as tile

    f16 = mybir.dt.float16
    f32 = mybir.dt.float32
    NB = NODE_BLOCK
    nc = bacc.Bacc("TRN2", target_bir_lowering=False, debug=False)

    nblocks = N_BLOCKS
    # Partition-major staging: xa[blk, p, c*NB+n] = slab c (of 0..3), row p,
    # node n. One contiguous 8000 B run per partition per block -> near
    # line-rate DMA descriptors (vs 2000 B with a slab-major layout).
    xa = nc.dram_tensor("xa", [nblocks, 128, 4 * NB], f16, kind="ExternalInput").ap()
    xb = nc.dram_tensor("xb", [nblocks, 128, 3 * NB], f16, kind="ExternalInput").ap()
    # t4 (fifth l=2 component, 64 rows) packed [128, NC_NODES/2]: node n<12500
    # on partitions 0:64 col n, else partitions 64:128 col n-12500.
    xt4 = nc.dram_tensor("xt4", [128, NC_NODES // 2], f16, kind="ExternalInput").ap()
    w0a = nc.dram_tensor("w0a", [128, 128], f16, kind="ExternalInput").ap()
    w0b = nc.dram_tensor("w0b", [128, 128], f16, kind="ExternalInput").ap()
    w1d = nc.dram_tensor("w1d", [128, 64], f16, kind="ExternalInput").ap()
    w2p = nc.dram_tensor("w2p", [128, 64], f16, kind="ExternalInput").ap()
    w2s = nc.dram_tensor("w2s", [128, 32], f16, kind="ExternalInput").ap()
    b0d = nc.dram_tensor("b0d", [128, 1], f32, kind="ExternalInput").ap()
    outa = nc.dram_tensor("outa", [nblocks, 128, 3 * NB], f16, kind="ExternalOutput").ap()
    outb = nc.dram_tensor("outb", [nblocks, 96, NB], f16, kind="ExternalOutput").ap()

    with tile.TileContext(nc) as tc:
        with (
            tc.tile_pool(name="wpool", bufs=1) as wpool,
            tc.tile_pool(name="t4pool", bufs=1) as t4pool,
            tc.tile_pool(name="inpool", bufs=4) as inpool,
            tc.tile_pool(name="psum", bufs=2, space="PSUM") as psum,
            tc.tile_pool(name="outpool", bufs=3) as outpool,
        ):
            wa_t = wpool.tile([128, 128], f16)
            wb_t = wpool.tile([128, 128], f16)
            w1_t = wpool.tile([128, 64], f16)
            w2p_t = wpool.tile([128, 64], f16)
            w2s_t = wpool.tile([128, 32], f16)
            b0_t = wpool.tile([128, 1], f32)
            nc.gpsimd.dma_start(wa_t[:], w0a)
            nc.gpsimd.dma_start(wb_t[:], w0b)
            nc.gpsimd.dma_start(w1_t[:], w1d)
            nc.gpsimd.dma_start(w2p_t[:], w2p)
            nc.gpsimd.dma_start(w2s_t[:], w2s)
            nc.gpsimd.dma_start(b0_t[:], b0d)

            # Stage the whole t4 slab once (3.2 MB) on the store ring before
            # stores start flowing; per-block loads stream on the HWDGE rings.
            t4_t = t4pool.tile([128, NC_NODES // 2], f16)
            nc.gpsimd.dma_start(t4_t[:], xt4)

            for blk in range(nblocks):
                tina = inpool.tile([128, 4 * NB], f16)
                tinb = inpool.tile([128, 3 * NB], f16)
                nc.sync.dma_start(tina[:], xa[blk])
                nc.scalar.dma_start(tinb[:], xb[blk])

                tout = outpool.tile([128, 4 * NB], f16)
                for h in range(2):
                    lo = blk * NB + h * HALF          # node offset of this half
                    sl = slice(h * HALF, h * HALF + HALF)
                    # t4 columns for this half (see xt4 packing above)
                    t4c = lo % (NC_NODES // 2)
                    t4p = 0 if lo < NC_NODES // 2 else 64
                    t4_mv = t4_t[t4p:t4p + 64, t4c:t4c + HALF]

                    p0 = psum.tile([128, HALF], f32)
                    p1 = psum.tile([128, HALF], f32)
                    p2 = psum.tile([128, HALF], f32)
                    p3 = psum.tile([96, HALF], f32)

                    def a(c):  # column slice of slab c in tina
                        return tina[:, c * NB + h * HALF: c * NB + h * HALF + HALF]

                    def b(c):
                        return tinb[:, c * NB + h * HALF: c * NB + h * HALF + HALF]

                    nc.tensor.matmul(p0[:], wa_t[:], a(0), start=True, stop=False)
                    nc.tensor.matmul(p0[:], wb_t[:], a(1), start=False, stop=True)
                    nc.tensor.matmul(p1[0:64, :], w1_t[:], a(2))
                    nc.tensor.matmul(p1[64:128, :], w1_t[:], a(3))
                    nc.tensor.matmul(p2[0:64, :], w1_t[:], b(0))
                    nc.tensor.matmul(p2[64:128, :], w2p_t[:], b(1))
                    nc.tensor.matmul(p3[0:64, :], w2p_t[:], b(2))
                    nc.tensor.matmul(p3[64:96, :], w2s_t[t4p:t4p + 64, :], t4_mv)

                    nc.vector.tensor_scalar_add(tout[:, sl], p0[:], b0_t[:])
                    nc.vector.tensor_copy(
                        tout[:, NB + h * HALF: NB + h * HALF + HALF], p1[:])
                    nc.vector.tensor_copy(
                        tout[:, 2 * NB + h * HALF: 2 * NB + h * HALF + HALF], p2[:])
                    nc.vector.tensor_copy(
                        tout[0:96, 3 * NB + h * HALF: 3 * NB + h * HALF + HALF], p3[:])

                nc.gpsimd.dma_start(outa[blk], tout[:, 0:3 * NB])
                nc.gpsimd.dma_start(outb[blk], tout[0:96, 3 * NB:4 * NB])

    nc.compile()
    return nc


def _get_program():
    if "p" not in _PROGRAM_CACHE:
        _PROGRAM_CACHE["p"] = _build_program()
    return _PROGRAM_CACHE["p"]


def _repack_inputs(x1, x2):
    """Build XR [896, N] fp16 (slabs s1,s2,v0,v1,v2,tp0,tp1) + T4 [64, N] fp16.

    Row slabs of XR: [s1] [s2] [v1_0|v2_0] [v1_1|v2_1] [v1_2|v2_2]
    [t_0|t_1] [t_2|t_3]; each t_i = [t1_i(32); t2_i(32)]. T4 = t_4.
    """
    n = x1.shape[0]
    xr = np.empty((896, n), dtype=np.float16)
    xr[0:128] = x1[:, 0:128].T
    xr[128:256] = x2[:, 0:128].T
    v1 = x1[:, 128:320].reshape(n, MUL1, 3)
    v2 = x2[:, 128:320].reshape(n, MUL1, 3)
    for i in range(3):
        base = 256 + 128 * i
        xr[base:base + 64] = v1[:, :, i].T
        xr[base + 64:base + 128] = v2[:, :, i].T
    t1 = x1[:, 320:480].reshape(n, MUL2, 5)
    t2 = x2[:, 320:480].reshape(n, MUL2, 5)
    for i in range(4):
        base = 640 + 64 * i
        xr[base:base + 32] = t1[:, :, i].T
        xr[base + 32:base + 64] = t2[:, :, i].T
    t4 = np.empty((64, n), dtype=np.float16)
    t4[0:32] = t1[:, :, 4].T
    t4[32:64] = t2[:, :, 4].T
    return xr, t4


def _prepare_in_maps(x1, x2, W0, W1, W2, b0):
    x1 = np.asarray(x1, dtype=np.float32)
    x2 = np.asarray(x2, dtype=np.float32)
    inv0 = np.float32(1.0 / np.sqrt(2 * MUL0))
    inv1 = np.float32(1.0 / np.sqrt(2 * MUL1))
    inv2 = np.float32(1.0 / np.sqrt(2 * MUL2))
    w0s = np.asarray(W0, np.float32) * inv0                            # [256, 128]
    w1s = np.asarray(W1, np.float32) * inv1                            # [128, 64]
    w2s = np.asarray(W2, np.float32) * inv2                            # [64, 32]
    w2pair = np.zeros((128, 64), dtype=np.float32)                     # blockdiag(W2s, W2s)
    w2pair[0:64, 0:32] = w2s
    w2pair[64:128, 32:64] = w2s
    w2stack = np.concatenate([w2s, w2s], axis=0)                       # [128, 32]
    weights = {
        "w0a": np.ascontiguousarray(w0s[0:128]).astype(np.float16),
        "w0b": np.ascontiguousarray(w0s[128:256]).astype(np.float16),
        "w1d": w1s.astype(np.float16),
        "w2p": w2pair.astype(np.float16),
        "w2s": w2stack.astype(np.float16),
        "b0d": np.ascontiguousarray(np.asarray(b0, np.float32).reshape(128, 1)),
    }
    xr, t4 = _repack_inputs(x1, x2)
    in_maps = []
    half = NC_NODES // 2
    for c in range(N_CORES):
        xrc = xr[:, c * NC_NODES:(c + 1) * NC_NODES]
        # Partition-major: [nblocks, 128, nchunks*NODE_BLOCK], per-partition
        # data contiguous within each block.
        xcb = xrc.reshape(7, 128, N_BLOCKS, NODE_BLOCK)
        xab = np.ascontiguousarray(
            xcb[0:4].transpose(2, 1, 0, 3).reshape(N_BLOCKS, 128, 4 * NODE_BLOCK)
        )
        xbb = np.ascontiguousarray(
            xcb[4:7].transpose(2, 1, 0, 3).reshape(N_BLOCKS, 128, 3 * NODE_BLOCK)
        )
        t4c = t4[:, c * NC_NODES:(c + 1) * NC_NODES]
        xt4 = np.ascontiguousarray(
            t4c.reshape(64, 2, half).transpose(1, 0, 2).reshape(128, half)
        )
        in_maps.append({"xa": xab, "xb": xbb, "xt4": xt4, **weights})
    return in_maps


def _assemble_output(outs):
    """outs: list of 8 (outa [nb,128,3*NB], outb [nb,96,NB]) -> [N_TOTAL, 480]."""
    full = np.empty((N_TOTAL, 480), dtype=np.float32)
    for c, (oa, ob) in enumerate(outs):
        oa = np.asarray(oa, np.float32).reshape(N_BLOCKS, 128, 3, NODE_BLOCK)
        o = np.empty((480, NC_NODES), dtype=np.float32)
        o[0:384] = (
            oa.transpose(2, 1, 0, 3).reshape(384, NC_NODES)
        )
        o[384:480] = (
            np.asarray(ob, np.float32).transpose(1, 0, 2).reshape(96, NC_NODES)
        )
        rows = slice(c * NC_NODES, (c + 1) * NC_NODES)
        full[rows, 0:128] = o[0:128].T
        full[rows, 128:320] = (
            o[128:320].reshape(3, MUL1, NC_NODES).transpose(2, 1, 0).reshape(NC_NODES, 192)
        )
        full[rows, 320:480] = (
            o[320:480].reshape(5, MUL2, NC_NODES).transpose(2, 1, 0).reshape(NC_NODES, 160)
        )
    return full


def kernel(x1, x2, W0, W1, W2, b0):
    from concourse.bass_utils import run_bass_kernel_spmd

    in_maps = _prepare_in_maps(x1, x2, W0, W1, W2, b0)
    nc = _get_program()
    res = run_bass_kernel_spmd(nc, in_maps, core_ids=list(range(N_CORES)))
    return _assemble_output([(r["outa"], r["outb"]) for r in res.results])
